# revision 33
# baseline (speedup 1.0000x reference)
"""Autoregressive GRU on 8 TRN2 NeuronCores.

Data-parallel: batch B=512 is split as 64 rows per core; the small GRU
weights are replicated and the T=128 sequential loop runs locally per core.

Key algebra (Keras GRU, reset_after=True, gate order [z, r, h]):
  step 0:  inp = 0, h = x  ->  gx = b[0], gh = x @ U + b[1]
  step t>=1: inp == h      ->  gx + gh uses (W + U) for the z and r gates
so per step we need ONE matmul against a host-prefused weight matrix:
  V  = [Wr+Ur | Uh | Wh | Wz+Uz]   (steps >= 1)   [D, 4D]
  V0 = [Ur   | Uh | 0  | Uz    ]   (step 0)       [D, 4D]
with PSUM bank layout [rpre | hh | xh | zpre], then
  r = sigmoid(rpre); hhat = tanh(xh + r*hh); z = sigmoid(zpre)
  h_new = hhat + z*(h - hhat)

Perf structure (what made this fast):
- float32r matmuls: fp32 operand storage at bf16 stream rate (1 cycle/row at
  N=512), so the fused weights carry no quantization error; only the bf16
  recurrent state and gate intermediates contribute (~1e-2 rel overall).
- One PSUM tile PER GATE BANK, ordered [r, hh, xh, z]: Tile's dependency
  tracking is tile-granular, so sigmoid(r) starts right after the r bank's
  4-matmul accumulation group instead of after all 16 matmuls, and the
  r -> p -> q -> tanh chain overlaps the rest of the matmul stream.
- hT (the next step's stationary operand) is rebuilt from TWO groups of PE
  transposes, exploiting linearity of the transpose:
      hT = copy(hhat^T) ; hT += tt^T      (DVE, SBUF + PSUM operands)
  The hhat^T group runs mid-tail on the otherwise-idle PE (which also keeps
  the HAM activity monitor at K=8/8 - otherwise the per-step idle window
  re-throttles the PE to 1.2 GHz and everything runs 2x slow), and only
  tt^T + the copy/add pair remain on the recurrence-critical chain. The
  batch-major h_new add, the f32 output copy and the output DMA all run
  off-chain. (Transpose-mode matmuls do NOT accumulate in PSUM - start/stop
  flags are ignored - hence the copy+add assembly on the DVE instead.)
- A warm-up preamble of identity matmuls (no DMA dependence) flips the PE
  clock gate to K=8/8 before step 0, and two tiny regular matmuls reading
  hhat/z anchor PE activity mid-tail. (~6% of matmuls still start at the
  1.2 GHz cold clock from HAM micro-oscillation across the PSUM
  accumulation groups; no filler strategy fixes it - transpose-mode ops
  are invisible to the activity monitor, and heavy regular-matmul filler
  tips the chip into the P0 power state, 2.4 -> 2.0 GHz on everything.)
- Measured converged step = 6.5-7.1 us: 2.97 us r/hh/xh matmul stream
  (overlapping sigmoid(r) -> p) + 0.7 q + 0.8 tanh + 0.67 sigmoid(z) +
  0.49 tt + 0.35 tt^T + 0.47 hT assembly + sem hops. Pairwise collectives
  measure ~9 us each on this fabric, so cross-core gate splitting with a
  per-step h exchange can never pay.
"""

import numpy as np
import ml_dtypes

B, D, T = 512, 512, 128
NCORES = 8
BLOC = B // NCORES  # 64
P = 128
KC = D // P  # 4 K-chunks
GW = 4 * D  # 2048 gate columns: [r | hh | xh | z]

_BF16 = ml_dtypes.bfloat16

# set by test harness to capture a profile; harmless when False
TRACE = False
TMPDIR = None
LAST = {}


def _prepare_weights(W, U, b):
    """Host-side fusion. Returns (V, V0, bias) in math layout."""
    Wz, Wr, Wh = W[:, :D], W[:, D : 2 * D], W[:, 2 * D :]
    Uz, Ur, Uh = U[:, :D], U[:, D : 2 * D], U[:, 2 * D :]
    V = np.concatenate([Wr + Ur, Uh, Wh, Wz + Uz], axis=1)  # [D, GW]
    V0 = np.concatenate([Ur, Uh, np.zeros_like(Wh), Uz], axis=1)
    b0, b1 = b[0], b[1]
    bias = np.concatenate(
        [b0[D : 2 * D] + b1[D : 2 * D], b1[2 * D :], b0[2 * D :], b0[:D] + b1[:D]]
    )  # [GW], order [r | hh | xh | z]
    return V, V0, bias


def _dev_layout(V):
    # V_dev[p, k*GW + j] = V[k*128 + p, j]
    return np.ascontiguousarray(
        V.reshape(KC, P, GW).transpose(1, 0, 2).reshape(P, KC * GW)
    )


_CACHE = {}


def _build(has_bias: bool):
    import concourse.mybir as mybir
    import concourse.tile as tile
    from concourse import bacc
    from concourse.masks import make_identity

    f32 = mybir.dt.float32
    f32r = mybir.dt.float32r
    bf16 = mybir.dt.bfloat16
    AF = mybir.ActivationFunctionType

    nc = bacc.Bacc(
        "TRN2", target_bir_lowering=False, debug=False, num_devices=NCORES
    )
    v0_d = nc.dram_tensor("v0", [P, KC * GW], f32r, kind="ExternalInput").ap()
    v_d = nc.dram_tensor("v", [P, KC * GW], f32r, kind="ExternalInput").ap()
    h0_d = nc.dram_tensor("h0", [BLOC, D], bf16, kind="ExternalInput").ap()
    h0T_d = nc.dram_tensor("h0T", [P, KC * BLOC], f32r, kind="ExternalInput").ap()
    if has_bias:
        bias_d = nc.dram_tensor("bias", [BLOC, GW], f32, kind="ExternalInput").ap()
    out_d = nc.dram_tensor("out", [BLOC, T, D], f32, kind="ExternalOutput").ap()

    with tile.TileContext(nc) as tc:
        with (
            tc.tile_pool(name="const", bufs=1) as cpool,
            tc.tile_pool(name="state", bufs=2) as spool,
            tc.tile_pool(name="work", bufs=3) as wpool,
            tc.tile_pool(name="outp", bufs=3) as opool,
            tc.tile_pool(name="gates", bufs=1, space="PSUM") as gpool,
            tc.tile_pool(name="trp", bufs=2, space="PSUM") as trpool,
            tc.tile_pool(name="warm", bufs=1, space="PSUM") as warmpool,
            tc.tile_pool(name="scr", bufs=1, space="PSUM") as scrpool,
        ):
            # weights as one tile per K-chunk: a step's k-th matmul group
            # only waits for its own chunk's DMA, not the whole 4 MB tensor
            v0_sb = [
                cpool.tile([P, GW], f32r, tag=f"v0{k}", name=f"v0{k}")
                for k in range(KC)
            ]
            v_sb = [
                cpool.tile([P, GW], f32r, tag=f"v{k}", name=f"v{k}")
                for k in range(KC)
            ]
            ident = cpool.tile([BLOC, BLOC], bf16, tag="ident")
            for k in range(KC):
                nc.sync.dma_start(v0_sb[k][:], v0_d[:, k * GW : (k + 1) * GW])
            make_identity(nc, ident[:])

            h = spool.tile([BLOC, D], bf16, tag="h")
            hT = spool.tile([P, KC * BLOC], f32r, tag="hT")
            nc.sync.dma_start(h[:], h0_d[:])
            nc.sync.dma_start(hT[:], h0T_d[:])
            for k in range(KC):
                nc.sync.dma_start(v_sb[k][:], v_d[:, k * GW : (k + 1) * GW])
            if has_bias:
                bias_sb = cpool.tile([BLOC, GW], f32, tag="bias")
                nc.sync.dma_start(bias_sb[:], bias_d[:])

            # PE warm-up: dense transpose work that depends only on the
            # locally-built identity (not on any DMA) flips the HAM clock
            # gate to K=8/8 while the weight DMAs are still in flight.
            wu = scrpool.tile([P, BLOC], f32, tag="scr", name="wu")
            for i in range(24):
                nc.tensor.matmul(
                    wu[:BLOC, :],
                    ident[:],
                    ident[:],
                    start=True,
                    stop=True,
                )

            for t in range(T):
                vsb = v0_sb if t == 0 else v_sb
                last = t == T - 1
                # one PSUM tile per gate bank: [r | hh | xh | z]
                gb = [
                    gpool.tile([BLOC, 512], f32, tag=f"g{n}", name=f"g{n}")
                    for n in range(4)
                ]
                def bank_mms(n, stop=True):
                    for k in range(KC):
                        nc.tensor.matmul(
                            gb[n][:],
                            hT[:, k * BLOC : (k + 1) * BLOC],
                            vsb[k][:, n * 512 : (n + 1) * 512],
                            start=(k == 0),
                            stop=(k == KC - 1) and stop,
                        )
                    if has_bias:
                        nc.vector.tensor_add(
                            gb[n][:], gb[n][:], bias_sb[:, n * 512 : (n + 1) * 512]
                        )

                bank_mms(0)  # rpre
                r = wpool.tile([BLOC, D], bf16, tag="r", name="r")
                nc.scalar.activation(r[:], gb[0][:], AF.Sigmoid)
                bank_mms(1)  # hh
                p = wpool.tile([BLOC, D], bf16, tag="p", name="p")
                nc.vector.tensor_mul(p[:], r[:], gb[1][:])
                bank_mms(2)  # xh
                q = wpool.tile([BLOC, D], bf16, tag="q", name="q")
                nc.vector.tensor_add(q[:], p[:], gb[2][:])
                bank_mms(3)  # zpre
                hhat = wpool.tile([BLOC, D], bf16, tag="hhat", name="hhat")
                nc.scalar.activation(hhat[:], q[:], AF.Tanh)

                if not last:
                    # trpA = hhat^T: real mid-tail PE activity (keeps the HAM
                    # clock gate warm) that feeds the hT rebuild below
                    trpA = warmpool.tile(
                        [P, KC * BLOC], bf16, tag="warm", name="trpA"
                    )
                    for k in range(KC):
                        nc.tensor.matmul(
                            trpA[:, k * BLOC : (k + 1) * BLOC],
                            hhat[:, k * P : (k + 1) * P],
                            ident[:],
                            is_transpose=True,
                            start=True,
                            stop=True,
                        )

                s = wpool.tile([BLOC, D], bf16, tag="s", name="s")
                nc.vector.tensor_sub(s[:], h[:], hhat[:])
                if not last:
                    # one tiny REGULAR matmul mid-tail: transpose-mode ops are
                    # invisible to the HAM activity monitor, so this (cheap,
                    # N=64) real matmul is what actually keeps K=8/8
                    scrf = scrpool.tile([P, BLOC], f32, tag="scr", name="scrf")
                    nc.tensor.matmul(
                        scrf[:], hhat[:, :P], ident[:], start=True, stop=True
                    )
                z = wpool.tile([BLOC, D], bf16, tag="z", name="z")
                nc.scalar.activation(z[:], gb[3][:], AF.Sigmoid)
                if not last:
                    nc.tensor.matmul(
                        scrf[:], z[:, :P], ident[:], start=True, stop=True
                    )
                tt = wpool.tile([BLOC, D], bf16, tag="t", name="tt")
                nc.vector.tensor_mul(tt[:], z[:], s[:])

                if not last:
                    # trpB = tt^T; then hT_new = trpA^ + trpB^ = h_new^T
                    # (transpose is linear), so the h_new add, the f32 output
                    # copy and the DMA all run OFF the recurrence chain
                    trpB = trpool.tile([P, KC * BLOC], bf16, tag="trp", name="trpB")
                    for k in range(KC):
                        nc.tensor.matmul(
                            trpB[:, k * BLOC : (k + 1) * BLOC],
                            tt[:, k * P : (k + 1) * P],
                            ident[:],
                            is_transpose=True,
                            start=True,
                            stop=True,
                        )
                    hT_new = spool.tile([P, KC * BLOC], f32r, tag="hT")
                    nc.vector.tensor_copy(hT_new[:], trpA[:])
                    nc.vector.tensor_add(hT_new[:], hT_new[:], trpB[:])
                    hT = hT_new

                h_new = spool.tile([BLOC, D], bf16, tag="h")
                nc.vector.tensor_add(h_new[:], hhat[:], tt[:])
                of = opool.tile([BLOC, D], f32, tag="of", name="of")
                nc.scalar.copy(of[:], h_new[:])
                nc.sync.dma_start(out_d[:, t, :], of[:])
                h = h_new

    nc.compile()
    return nc


def kernel(x, W, U, b):
    from concourse.bass_utils import run_bass_kernel_spmd

    x = np.asarray(x, dtype=np.float32)
    W = np.asarray(W, dtype=np.float32)
    U = np.asarray(U, dtype=np.float32)
    b = np.asarray(b, dtype=np.float32)

    V, V0, bias = _prepare_weights(W, U, b)
    has_bias = bool(np.any(bias != 0.0))
    v_dev = _dev_layout(V).astype(np.float32)
    v0_dev = _dev_layout(V0).astype(np.float32)

    key = ("gru", has_bias)
    if key not in _CACHE:
        _CACHE[key] = _build(has_bias)
    nc = _CACHE[key]

    in_maps = []
    for i in range(NCORES):
        xs = x[i * BLOC : (i + 1) * BLOC]  # [64, 512]
        m = {
            "v0": v0_dev,
            "v": v_dev,
            "h0": xs.astype(_BF16),
            "h0T": np.ascontiguousarray(
                xs.astype(_BF16)
                .astype(np.float32)
                .reshape(BLOC, KC, P)
                .transpose(2, 1, 0)
                .reshape(P, KC * BLOC)
            ),
        }
        if has_bias:
            m["bias"] = np.ascontiguousarray(
                np.broadcast_to(bias[None, :], (BLOC, GW))
            ).astype(np.float32)
        in_maps.append(m)

    res = run_bass_kernel_spmd(
        nc, in_maps, core_ids=list(range(NCORES)), trace=TRACE, tmpdir=TMPDIR
    )
    LAST["exec_time_ns"] = res.exec_time_ns
    LAST["results"] = res
    out = np.concatenate([res.results[i]["out"] for i in range(NCORES)], axis=0)
    return out.astype(np.float32)


# revision 35
# speedup vs baseline: 1.0036x; 1.0036x over previous
"""Autoregressive GRU on 8 TRN2 NeuronCores.

Data-parallel: batch B=512 is split as 64 rows per core; the small GRU
weights are replicated and the T=128 sequential loop runs locally per core.

Key algebra (Keras GRU, reset_after=True, gate order [z, r, h]):
  step 0:  inp = 0, h = x  ->  gx = b[0], gh = x @ U + b[1]
  step t>=1: inp == h      ->  gx + gh uses (W + U) for the z and r gates
so per step we need ONE matmul against a host-prefused weight matrix:
  V  = [Wr+Ur | Uh | Wh | Wz+Uz]   (steps >= 1)   [D, 4D]
  V0 = [Ur   | Uh | 0  | Uz    ]   (step 0)       [D, 4D]
with PSUM bank layout [rpre | hh | xh | zpre], then
  r = sigmoid(rpre); hhat = tanh(xh + r*hh); z = sigmoid(zpre)
  h_new = hhat + z*(h - hhat)

Perf structure (what made this fast):
- float32r matmuls: fp32 operand storage at bf16 stream rate (1 cycle/row at
  N=512), so the fused weights carry no quantization error; only the bf16
  recurrent state and gate intermediates contribute (~1e-2 rel overall).
- One PSUM tile PER GATE BANK, ordered [r, hh, xh, z]: Tile's dependency
  tracking is tile-granular, so sigmoid(r) starts right after the r bank's
  4-matmul accumulation group instead of after all 16 matmuls, and the
  r -> p -> q -> tanh chain overlaps the rest of the matmul stream.
- hT (the next step's stationary operand) is rebuilt from TWO groups of PE
  transposes, exploiting linearity of the transpose:
      hT = copy(hhat^T) ; hT += tt^T      (DVE, SBUF + PSUM operands)
  The hhat^T group runs mid-tail on the otherwise-idle PE (which also keeps
  the HAM activity monitor at K=8/8 - otherwise the per-step idle window
  re-throttles the PE to 1.2 GHz and everything runs 2x slow), and only
  tt^T + the copy/add pair remain on the recurrence-critical chain. The
  batch-major h_new add, the f32 output copy and the output DMA all run
  off-chain. (Transpose-mode matmuls do NOT accumulate in PSUM - start/stop
  flags are ignored - hence the copy+add assembly on the DVE instead.)
- A warm-up preamble of identity matmuls (no DMA dependence) flips the PE
  clock gate to K=8/8 before step 0, and two tiny regular matmuls reading
  hhat/z anchor PE activity mid-tail. (~6% of matmuls still start at the
  1.2 GHz cold clock from HAM micro-oscillation across the PSUM
  accumulation groups; no filler strategy fixes it - transpose-mode ops
  are invisible to the activity monitor, and heavy regular-matmul filler
  tips the chip into the P0 power state, 2.4 -> 2.0 GHz on everything.)
- Measured converged step = 6.5-7.1 us: 2.97 us r/hh/xh matmul stream
  (overlapping sigmoid(r) -> p) + 0.7 q + 0.8 tanh + 0.67 sigmoid(z) +
  0.49 tt + 0.35 tt^T + 0.47 hT assembly + sem hops. Pairwise collectives
  measure ~9 us each on this fabric, so cross-core gate splitting with a
  per-step h exchange can never pay.
"""

import numpy as np
import ml_dtypes

B, D, T = 512, 512, 128
NCORES = 8
BLOC = B // NCORES  # 64
P = 128
KC = D // P  # 4 K-chunks
GW = 4 * D  # 2048 gate columns: [r | hh | xh | z]

_BF16 = ml_dtypes.bfloat16

# set by test harness to capture a profile; harmless when False
TRACE = False
TMPDIR = None
LAST = {}


def _prepare_weights(W, U, b):
    """Host-side fusion. Returns (V, V0, bias) in math layout."""
    Wz, Wr, Wh = W[:, :D], W[:, D : 2 * D], W[:, 2 * D :]
    Uz, Ur, Uh = U[:, :D], U[:, D : 2 * D], U[:, 2 * D :]
    V = np.concatenate([Wr + Ur, Uh, Wh, Wz + Uz], axis=1)  # [D, GW]
    V0 = np.concatenate([Ur, Uh, np.zeros_like(Wh), Uz], axis=1)
    b0, b1 = b[0], b[1]
    bias = np.concatenate(
        [b0[D : 2 * D] + b1[D : 2 * D], b1[2 * D :], b0[2 * D :], b0[:D] + b1[:D]]
    )  # [GW], order [r | hh | xh | z]
    return V, V0, bias


def _dev_layout(V):
    # V_dev[p, k*GW + j] = V[k*128 + p, j]
    return np.ascontiguousarray(
        V.reshape(KC, P, GW).transpose(1, 0, 2).reshape(P, KC * GW)
    )


_CACHE = {}


def _build(has_bias: bool):
    import concourse.mybir as mybir
    import concourse.tile as tile
    from concourse import bacc
    from concourse.masks import make_identity

    f32 = mybir.dt.float32
    f32r = mybir.dt.float32r
    bf16 = mybir.dt.bfloat16
    AF = mybir.ActivationFunctionType

    nc = bacc.Bacc(
        "TRN2", target_bir_lowering=False, debug=False, num_devices=NCORES
    )
    v0_d = nc.dram_tensor("v0", [P, KC * GW], f32r, kind="ExternalInput").ap()
    v_d = nc.dram_tensor("v", [P, KC * GW], f32r, kind="ExternalInput").ap()
    h0_d = nc.dram_tensor("h0", [BLOC, D], bf16, kind="ExternalInput").ap()
    h0T_d = nc.dram_tensor("h0T", [P, KC * BLOC], f32r, kind="ExternalInput").ap()
    if has_bias:
        bias_d = nc.dram_tensor("bias", [BLOC, GW], f32, kind="ExternalInput").ap()
    out_d = nc.dram_tensor("out", [BLOC, T, D], f32, kind="ExternalOutput").ap()

    with tile.TileContext(nc) as tc:
        with (
            tc.tile_pool(name="const", bufs=1) as cpool,
            tc.tile_pool(name="state", bufs=2) as spool,
            tc.tile_pool(name="work", bufs=3) as wpool,
            tc.tile_pool(name="outp", bufs=3) as opool,
            tc.tile_pool(name="gates", bufs=1, space="PSUM") as gpool,
            tc.tile_pool(name="trp", bufs=2, space="PSUM") as trpool,
            tc.tile_pool(name="warm", bufs=1, space="PSUM") as warmpool,
            tc.tile_pool(name="scr", bufs=1, space="PSUM") as scrpool,
        ):
            v0_sb = cpool.tile([P, KC * GW], f32r, tag="v0")
            v_sb = cpool.tile([P, KC * GW], f32r, tag="v")
            ident = cpool.tile([BLOC, BLOC], bf16, tag="ident")
            nc.sync.dma_start(v0_sb[:], v0_d[:])
            make_identity(nc, ident[:])

            h = spool.tile([BLOC, D], bf16, tag="h")
            hT = spool.tile([P, KC * BLOC], f32r, tag="hT")
            nc.sync.dma_start(h[:], h0_d[:])
            nc.sync.dma_start(hT[:], h0T_d[:])
            nc.sync.dma_start(v_sb[:], v_d[:])
            if has_bias:
                bias_sb = cpool.tile([BLOC, GW], f32, tag="bias")
                nc.sync.dma_start(bias_sb[:], bias_d[:])

            # PE warm-up: dense transpose work that depends only on the
            # locally-built identity (not on any DMA) flips the HAM clock
            # gate to K=8/8 while the weight DMAs are still in flight.
            wu = trpool.tile([P, KC * BLOC], bf16, tag="trp", name="wu")
            for i in range(24):
                nc.tensor.matmul(
                    wu[:BLOC, (i % KC) * BLOC : (i % KC + 1) * BLOC],
                    ident[:],
                    ident[:],
                    is_transpose=True,
                    start=True,
                    stop=True,
                )

            for t in range(T):
                vsb = v0_sb if t == 0 else v_sb
                last = t == T - 1
                # one PSUM tile per gate bank: [r | hh | xh | z]
                gb = [
                    gpool.tile([BLOC, 512], f32, tag=f"g{n}", name=f"g{n}")
                    for n in range(4)
                ]
                def bank_mms(n, stop=True):
                    for k in range(KC):
                        nc.tensor.matmul(
                            gb[n][:],
                            hT[:, k * BLOC : (k + 1) * BLOC],
                            vsb[:, k * GW + n * 512 : k * GW + (n + 1) * 512],
                            start=(k == 0),
                            stop=(k == KC - 1) and stop,
                        )
                    if has_bias:
                        nc.vector.tensor_add(
                            gb[n][:], gb[n][:], bias_sb[:, n * 512 : (n + 1) * 512]
                        )

                bank_mms(0)  # rpre
                r = wpool.tile([BLOC, D], bf16, tag="r", name="r")
                nc.scalar.activation(r[:], gb[0][:], AF.Sigmoid)
                bank_mms(1)  # hh
                p = wpool.tile([BLOC, D], bf16, tag="p", name="p")
                nc.vector.tensor_mul(p[:], r[:], gb[1][:])
                bank_mms(2)  # xh
                # q goes into the retired r-gate PSUM bank (free after
                # sigmoid(r)/p consumed it): ScalarE reads PSUM faster than
                # SBUF, so tanh starts ~50-150 ns sooner
                q = gb[0]
                nc.vector.tensor_add(q[:], p[:], gb[2][:])
                bank_mms(3)  # zpre
                if not last:
                    scr = scrpool.tile([P, KC * BLOC], bf16, tag="scr", name="scr")
                    for k in range(3):
                        nc.tensor.matmul(
                            scr[:, k * BLOC : (k + 1) * BLOC],
                            p[:, k * P : (k + 1) * P],
                            ident[:],
                            is_transpose=True,
                            start=True,
                            stop=True,
                        )
                hhat = wpool.tile([BLOC, D], bf16, tag="hhat", name="hhat")
                nc.scalar.activation(hhat[:], q[:], AF.Tanh)

                if not last:
                    # trpA = hhat^T: real mid-tail PE activity (keeps the HAM
                    # clock gate warm) that feeds the hT rebuild below
                    trpA = warmpool.tile(
                        [P, KC * BLOC], bf16, tag="warm", name="trpA"
                    )
                    for k in range(KC):
                        nc.tensor.matmul(
                            trpA[:, k * BLOC : (k + 1) * BLOC],
                            hhat[:, k * P : (k + 1) * P],
                            ident[:],
                            is_transpose=True,
                            start=True,
                            stop=True,
                        )

                s = wpool.tile([BLOC, D], bf16, tag="s", name="s")
                nc.vector.tensor_sub(s[:], h[:], hhat[:])
                if not last:
                    for k in range(2):
                        nc.tensor.matmul(
                            scr[:, k * BLOC : (k + 1) * BLOC],
                            s[:, k * P : (k + 1) * P],
                            ident[:],
                            is_transpose=True,
                            start=True,
                            stop=True,
                        )
                z = wpool.tile([BLOC, D], bf16, tag="z", name="z")
                nc.scalar.activation(z[:], gb[3][:], AF.Sigmoid)
                if not last:
                    for k in range(2):
                        nc.tensor.matmul(
                            scr[:, (2 + k) * BLOC : (3 + k) * BLOC],
                            z[:, k * P : (k + 1) * P],
                            ident[:],
                            is_transpose=True,
                            start=True,
                            stop=True,
                        )
                tt = wpool.tile([BLOC, D], bf16, tag="t", name="tt")
                nc.vector.tensor_mul(tt[:], z[:], s[:])

                if not last:
                    # trpB = tt^T; then hT_new = trpA^ + trpB^ = h_new^T
                    # (transpose is linear), so the h_new add, the f32 output
                    # copy and the DMA all run OFF the recurrence chain
                    trpB = trpool.tile([P, KC * BLOC], bf16, tag="trp", name="trpB")
                    for k in range(KC):
                        nc.tensor.matmul(
                            trpB[:, k * BLOC : (k + 1) * BLOC],
                            tt[:, k * P : (k + 1) * P],
                            ident[:],
                            is_transpose=True,
                            start=True,
                            stop=True,
                        )
                    hT_new = spool.tile([P, KC * BLOC], f32r, tag="hT")
                    nc.vector.tensor_copy(hT_new[:], trpA[:])
                    nc.vector.tensor_add(hT_new[:], hT_new[:], trpB[:])
                    hT = hT_new

                h_new = spool.tile([BLOC, D], bf16, tag="h")
                nc.vector.tensor_add(h_new[:], hhat[:], tt[:])
                of = opool.tile([BLOC, D], f32, tag="of", name="of")
                nc.scalar.copy(of[:], h_new[:])
                nc.sync.dma_start(out_d[:, t, :], of[:])
                h = h_new

    nc.compile()
    return nc


def kernel(x, W, U, b):
    from concourse.bass_utils import run_bass_kernel_spmd

    x = np.asarray(x, dtype=np.float32)
    W = np.asarray(W, dtype=np.float32)
    U = np.asarray(U, dtype=np.float32)
    b = np.asarray(b, dtype=np.float32)

    V, V0, bias = _prepare_weights(W, U, b)
    has_bias = bool(np.any(bias != 0.0))
    v_dev = _dev_layout(V).astype(np.float32)
    v0_dev = _dev_layout(V0).astype(np.float32)

    key = ("gru", has_bias)
    if key not in _CACHE:
        _CACHE[key] = _build(has_bias)
    nc = _CACHE[key]

    in_maps = []
    for i in range(NCORES):
        xs = x[i * BLOC : (i + 1) * BLOC]  # [64, 512]
        m = {
            "v0": v0_dev,
            "v": v_dev,
            "h0": xs.astype(_BF16),
            "h0T": np.ascontiguousarray(
                xs.astype(_BF16)
                .astype(np.float32)
                .reshape(BLOC, KC, P)
                .transpose(2, 1, 0)
                .reshape(P, KC * BLOC)
            ),
        }
        if has_bias:
            m["bias"] = np.ascontiguousarray(
                np.broadcast_to(bias[None, :], (BLOC, GW))
            ).astype(np.float32)
        in_maps.append(m)

    res = run_bass_kernel_spmd(
        nc, in_maps, core_ids=list(range(NCORES)), trace=TRACE, tmpdir=TMPDIR
    )
    LAST["exec_time_ns"] = res.exec_time_ns
    LAST["results"] = res
    out = np.concatenate([res.results[i]["out"] for i in range(NCORES)], axis=0)
    return out.astype(np.float32)
